# revision 40
# baseline (speedup 1.0000x reference)
"""Bundle-adjustment projection kernel for 8 Trainium2 NeuronCores.

out[v, n, :] = (u, v) pixel projection of point n under view v
(reference: nn_BundleAdjustmentModel).

Sharding: data-parallel over POINTS — each core takes a 62500-point slab
(padded to 63488) and all 64 views. Per (view, point) the math is three
linear functionals of the point followed by a projective divide:

  Z = Mz.p - d             (z camera coord)
  P = (cx*Mz - f*R0).p + (cx*(-d) - f*tx)      so  u = P / Z
  Q = (cy*Mz + f*R1).p + (cy*(-d) + f*ty)      so  v = Q / Z
  rs = clip(1/Z, +-1e4)  ==  1/(sign(Z)*max(|Z|, 1e-4))  up to a
       negligible cx*(|Z|/eps - 1) <= cx error inside the clip region.

The linear part runs on the PE as 32x32 tile_position-packed matmuls:
stationary = per-view coefficient columns [K=21, M=32], moving = point
chunks [21, 512] in bf16.  fp32-grade accuracy comes from a 3-level bf16
split of both points and coefficients (hi/mid/lo, error ~2^-26):

  C.p ~= Ch.(ph+pm+pl) + Cm.(ph+pm) + Cl.ph + (bh+bm+bl)   -> 21 K-rows

Each round: 2 chunks x 3 types x 2 view-halves = 12 matmuls (emitted
type-major/chunk-inner so the in-order PE sequencer always finds a free
32x32 subarray) fill PSUM banks laid out [Z|Z] and [P|P|Q|Q] (two
chunks x 64 views each), so the elementwise tail runs at full
128-partition occupancy:

  ACT    pq16 = copy(PQ psum) -> fp16 sbuf        (enables DVE 2x TT)
  DVE    r16  = RECIP_CLIP_ANT(Zbank) -> fp16     (custom 8-slice op:
         1-NR bitwise-not reciprocal, ~0.17% max err, min/max clip
         fused -- keeps the whole divide chain on one engine)
  DVE    uv   = pq16 * broadcast(r16) -> bf16     (one 2x TT for both
         planes via a 0-stride plane dim on r16)
  DMA    batched over 4 rounds: [64 views, {U,V}, 2048 pts] 1KB rows

A ~4us burst of dependency-free warm-up matmuls during the input-DMA
prefix pushes the PE HAM activity window past its threshold so round 0
starts at 2.4 GHz.  Output is bf16 u/v planes (absmax-normalized error
~3e-3 << 2e-2 gate); host interleaves and upcasts to fp32.
"""
import sys
import types

import numpy as np

V = 64
N = 500000
NC = 8                  # cores
PTS_CORE = N // NC      # 62500 real points per core
CH = 512                # points per matmul chunk
QCH = 31                # chunks per row-group quarter
NQ = QCH * CH           # 15872 points per quarter
SLAB = 4 * NQ           # 63488 padded points per core
BATCH = 4               # same-parity rounds per output DMA batch
TTV_ON = "gpsimd"       # engine for the v-plane multiply
KROWS = 21
Z_EPS = 1e-4
MIN_FOCAL = 50.0
MIN_DISTANCE = 0.25

_CACHE = {}


def _setup_paths():
    if "/opt/trn_rl_repo" not in sys.path:
        sys.path.insert(0, "/opt/trn_rl_repo")
    # the axon trace path imports antenv.axon_hooks; provide a stub if absent
    try:
        import antenv
        if not hasattr(antenv, "axon_hooks"):
            mod = types.ModuleType("antenv.axon_hooks")
            mod._hook = None
            mod.set_axon_ntff_profile_hook = lambda h: setattr(mod, "_hook", h)
            mod.get_axon_ntff_profile_hook = lambda: mod._hook
            sys.modules["antenv.axon_hooks"] = mod
            antenv.axon_hooks = mod
    except ImportError:
        pass


def _register_recip_clip():
    """Custom DVE op: out = clip(approx(1/x), +-imm2), fp16-writable.

    1-NR variant of RECIPROCAL_APPROX_FAST (~0.17% max rel err) with the
    min/max clip fused into the freed pipeline stages, so the clip no
    longer bounces to GPSIMD between two DVE ops.
    """
    import numpy as np
    from concourse import dve_ops
    from concourse.dve_spec import (
        C0, C1, C2, AluOp, Bin, Spec, Src0, Zero, lower, maxx, minn,
    )
    from concourse.dve_uop import DveOpSpec

    name = "RECIP_CLIP_ANT"
    for op in dve_ops.OPS:
        if op.name == name:
            return op

    def ref(in0, in1, c0, c1, c2):
        not_x = (~in0.view(np.int32)).view(np.float32)
        y0 = not_x * c0
        y1 = y0 * (c1 - in0 * y0)
        return np.minimum(np.maximum(y1, -c2), c2)

    _not_x = Bin(AluOp.BITWISE_NOT, Src0, Src0)
    _y0 = _not_x * C0
    _y1 = _y0 * (C1 - Src0 * _y0)
    spec = Spec(body=minn(maxx(_y1, Zero - C2), C2), reference=ref)
    op = dve_ops.DveOp(name, spec, subdim=False, uops_sha={})
    dve_ops.OPS.append(op)
    dve_ops.CUSTOM_DVE_SPECS[name] = op.spec
    row = dve_ops._CUSTOM_DVE_ROW_BASE + len(dve_ops.OPS) - 1
    dve_ops._SUB_OPCODE_FOR_NAME[name] = row
    for ver in ("v3",):
        tmp = DveOpSpec(name=name, opcode=row, uops=lower(spec, ver=ver),
                        rd1_en=False)
        op.uops_sha[ver] = tmp.sha(ver)
    return op


RECIP_C = {"s0": -0.23549794, "s1": 2.00173235}


def _build_nc():
    import concourse.bacc as bacc
    import concourse.mybir as mybir
    from concourse import tile

    dt = mybir.dt
    ALU = mybir.AluOpType
    AF = mybir.ActivationFunctionType
    recip_clip = _register_recip_clip()

    nc = bacc.Bacc("TRN2", target_bir_lowering=False, debug=False)
    MOV = nc.dram_tensor("MOV", [128, NQ], dt.bfloat16, kind="ExternalInput")
    STAT = nc.dram_tensor("STAT", [128, 192], dt.bfloat16, kind="ExternalInput")
    OUT = nc.dram_tensor("OUT", [V, 2, SLAB], dt.bfloat16, kind="ExternalOutput")

    with tile.TileContext(nc) as tc:
        with (
            tc.tile_pool(name="inp", bufs=1) as ip,
            tc.tile_pool(name="psz", bufs=2, space="PSUM") as ppz,
            tc.tile_pool(name="pspq", bufs=3, space="PSUM") as ppq,
            tc.tile_pool(name="wrk", bufs=4) as wp,
            tc.tile_pool(name="uvp", bufs=3) as up,
        ):
            mov = ip.tile([128, NQ], dt.bfloat16)
            stat = ip.tile([128, 192], dt.bfloat16)
            # staged loads: a small head chunk unblocks round 0 fast;
            # parity-1 data (partitions 64:128) can arrive much later
            HEAD = 4 * CH
            nc.sync.dma_start(out=mov[0:64, :HEAD], in_=MOV.ap()[0:64, :HEAD])
            nc.scalar.dma_start(out=stat[:], in_=STAT.ap())
            nc.sync.dma_start(out=mov[0:64, HEAD:], in_=MOV.ap()[0:64, HEAD:])
            nc.sync.dma_start(out=mov[64:128, :], in_=MOV.ap()[64:128, :])

            # PE warm-up: ~12 dependency-free matmuls during the DMA
            # prefix push the HAM activity window past its threshold, so
            # round 0 starts at 2.4 GHz instead of the cold 1.2 GHz.
            wtile = ip.tile([128, CH], dt.bfloat16)
            nc.gpsimd.memset(wtile[:], 0.0)
            for w in range(9):
                zw = ppz.tile([128, CH], dt.float32, name="Zw", tag="Z")
                for g in range(4):
                    nc.tensor.matmul(
                        out=zw[32 * g:32 * g + 32, :],
                        lhsT=wtile[32 * g:32 * g + KROWS, 0:32],
                        rhs=wtile[32 * g:32 * g + KROWS, :],
                        start=True, stop=True,
                        tile_position=(32 * g, 32 * g),
                    )

            # rounds: (parity, q).  Batches of BATCH same-parity rounds
            # share one uva output tile and one pair of output DMAs.
            rounds = [(par, q) for par in (0, 1) for q in range(QCH)]
            ttv = nc.gpsimd if TTV_ON == "gpsimd" else nc.vector

            uva = None
            for (par, q) in rounds:
                gA, gB = 2 * par, 2 * par + 1
                b = q % BATCH
                nb = min(BATCH, QCH - (q - b))
                Zb = ppz.tile([128, CH], dt.float32, name="Zb", tag="Z")
                PQb = ppq.tile([128, 2 * CH], dt.float32, name="PQb", tag="PQ")
                # type-major, chunk-inner order: consecutive MMs hit free
                # subarrays so the in-order NX never stalls on a busy one
                for t, banksl in ((0, Zb[:, :]),
                                  (1, PQb[:, 0:CH]),
                                  (2, PQb[:, CH:2 * CH])):
                    for (g, colbase) in ((gA, 0), (gB, 64)):
                        rhs = mov[32 * g:32 * g + KROWS, q * CH:(q + 1) * CH]
                        for h in (0, 1):
                            cb = colbase + 32 * h
                            nc.tensor.matmul(
                                out=banksl[cb:cb + 32, :],
                                lhsT=stat[32 * g:32 * g + KROWS,
                                          64 * t + 32 * h:
                                          64 * t + 32 * h + 32],
                                rhs=rhs,
                                start=True, stop=True,
                                tile_position=(32 * g, cb),
                            )
                pq16 = wp.tile([128, 2 * CH], dt.float16, name="pq16",
                               tag="pq16")
                nc.scalar.activation(pq16, PQb, AF.Copy)
                r16 = wp.tile([128, CH], dt.float16, name="r16", tag="r16")
                nc.vector._custom_dve(
                    recip_clip, out=r16, in0=Zb,
                    s0=RECIP_C["s0"], s1=RECIP_C["s1"], imm2=1.0 / Z_EPS)
                if b == 0:
                    uva = up.tile([128, 2, BATCH * CH], dt.bfloat16,
                                  name="uva", tag="uva")
                s = slice(b * CH, (b + 1) * CH)
                # one TT for both planes: r16 broadcast over the plane dim
                nc.vector.tensor_tensor(
                    uva[:, :, s],
                    pq16.rearrange("p (two n) -> p two n", two=2),
                    r16.rearrange("p (one n) -> p one n",
                                  one=1).broadcast_to([128, 2, CH]),
                    ALU.mult)
                if b == nb - 1:
                    j0 = q - nb + 1
                    w = nb * CH
                    for (g, pbase) in ((gA, 0), (gB, 64)):
                        pos = g * NQ + j0 * CH
                        nc.sync.dma_start(
                            out=OUT.ap()[:, :, pos:pos + w],
                            in_=uva[pbase:pbase + 64, :, :w],
                        )
    nc.compile()
    return nc


def _rotations_f32(euler):
    """Reference's fp32 rotation construction, in numpy."""
    euler = np.asarray(euler, np.float32)
    c = np.cos(euler)
    s = np.sin(euler)
    cx_, cy_, cz_ = c[:, 0], c[:, 1], c[:, 2]
    sx_, sy_, sz_ = s[:, 0], s[:, 1], s[:, 2]
    one = np.ones_like(cx_)
    zero = np.zeros_like(cx_)
    rx = np.stack([
        np.stack([one, zero, zero], -1),
        np.stack([zero, cx_, -sx_], -1),
        np.stack([zero, sx_, cx_], -1)], -2).astype(np.float32)
    ry = np.stack([
        np.stack([cy_, zero, sy_], -1),
        np.stack([zero, one, zero], -1),
        np.stack([-sy_, zero, cy_], -1)], -2).astype(np.float32)
    rz = np.stack([
        np.stack([cz_, -sz_, zero], -1),
        np.stack([sz_, cz_, zero], -1),
        np.stack([zero, zero, one], -1)], -2).astype(np.float32)
    return np.matmul(np.matmul(rx, ry), rz).astype(np.float32)


def _split3(a, BF):
    """float64 array -> 3-level bf16 decomposition (sum error ~2^-26)."""
    a = np.asarray(a, np.float64)
    h = a.astype(BF)
    r = a - h.astype(np.float64)
    m = r.astype(BF)
    r2 = r - m.astype(np.float64)
    lo = r2.astype(BF)
    return h, m, lo


def _host_precompute(euler, translation_xy, translation_depth_raw,
                     focal_raw, cx, cy, BF):
    """Per-view Z/P/Q coefficient stationary tile [128, 192] bf16."""
    rot = _rotations_f32(euler).astype(np.float64)        # [V,3,3]
    tdr = np.asarray(translation_depth_raw, np.float32)
    depth = (np.logaddexp(tdr, np.float32(0.0)).astype(np.float32)
             + np.float32(MIN_DISTANCE)).astype(np.float64)   # [V]
    fr = np.float32(np.asarray(focal_raw).reshape(-1)[0])
    focal = np.float64(
        np.float32(np.logaddexp(fr, np.float32(0.0))) + np.float32(MIN_FOCAL))
    txy = np.asarray(translation_xy, np.float32).astype(np.float64)
    cxf, cyf = np.float64(cx), np.float64(cy)

    Mz = rot[:, 2, :]                                      # [V,3]
    bz = -depth                                            # [V]
    Pc = cxf * Mz - focal * rot[:, 0, :]
    bp = cxf * bz - focal * txy[:, 0]
    Qc = cyf * Mz + focal * rot[:, 1, :]
    bq = cyf * bz + focal * txy[:, 1]

    stat = np.zeros((128, 192), np.float64)
    for t, (C, b) in enumerate(((Mz, bz), (Pc, bp), (Qc, bq))):
        Ch, Cm, Cl = _split3(C, BF)
        bh, bm, bl = _split3(b, BF)
        cols = slice(64 * t, 64 * t + 64)
        for g in range(4):
            r0 = 32 * g
            stat[r0 + 0:r0 + 3, cols] = Ch.astype(np.float64).T
            stat[r0 + 3:r0 + 6, cols] = Ch.astype(np.float64).T
            stat[r0 + 6:r0 + 9, cols] = Ch.astype(np.float64).T
            stat[r0 + 9:r0 + 12, cols] = Cm.astype(np.float64).T
            stat[r0 + 12:r0 + 15, cols] = Cm.astype(np.float64).T
            stat[r0 + 15:r0 + 18, cols] = Cl.astype(np.float64).T
            stat[r0 + 18, cols] = bh.astype(np.float64)
            stat[r0 + 19, cols] = bm.astype(np.float64)
            stat[r0 + 20, cols] = bl.astype(np.float64)
    return stat.astype(BF)


def _moving_for_slab(slab, BF):
    """[SLAB, 3] fp32 points -> MOV [128, NQ] bf16 (21 rows/quarter)."""
    mov = np.zeros((128, NQ), np.float64)
    for g in range(4):
        qpts = slab[g * NQ:(g + 1) * NQ].astype(np.float64)   # [NQ, 3]
        h, m, lo = _split3(qpts, BF)
        h = h.astype(np.float64).T    # [3, NQ]
        m = m.astype(np.float64).T
        lo = lo.astype(np.float64).T
        r0 = 32 * g
        mov[r0 + 0:r0 + 3] = h
        mov[r0 + 3:r0 + 6] = m
        mov[r0 + 6:r0 + 9] = lo
        mov[r0 + 9:r0 + 12] = h
        mov[r0 + 12:r0 + 15] = m
        mov[r0 + 15:r0 + 18] = h
        mov[r0 + 18:r0 + 21] = 1.0
    return mov.astype(BF)


def kernel(points, euler, translation_xy, translation_depth_raw, focal_raw,
           cx, cy, _trace=False):
    _setup_paths()
    import ml_dtypes
    from concourse.bass_utils import run_bass_kernel_spmd

    BF = ml_dtypes.bfloat16

    if "nc" not in _CACHE:
        _CACHE["nc"] = _build_nc()
    nc = _CACHE["nc"]

    points = np.ascontiguousarray(np.asarray(points, np.float32))
    stat = _host_precompute(euler, translation_xy, translation_depth_raw,
                            focal_raw, cx, cy, BF)

    in_maps = []
    for c in range(NC):
        slab = np.zeros((SLAB, 3), np.float32)
        slab[:PTS_CORE] = points[c * PTS_CORE:(c + 1) * PTS_CORE]
        in_maps.append({"MOV": _moving_for_slab(slab, BF), "STAT": stat})

    res = run_bass_kernel_spmd(nc, in_maps, list(range(NC)), trace=_trace)
    _CACHE["last_results"] = res

    out = np.empty((V, N, 2), np.float32)
    for c in range(NC):
        o = np.asarray(res.results[c]["OUT"])  # [V, 2, SLAB] bf16
        sl = slice(c * PTS_CORE, (c + 1) * PTS_CORE)
        out[:, sl, 0] = o[:, 0, :PTS_CORE].astype(np.float32)
        out[:, sl, 1] = o[:, 1, :PTS_CORE].astype(np.float32)
    return out
